# revision 1
# baseline (speedup 1.0000x reference)
"""Trainium2 Bass kernel for nn_CustomConv2D (degenerate conv: only the last
input channel contributes; 3x3 VALID conv -> 64 out channels + bias).

Strategy:
  - Host: slice x_padded[:, -1] (the only channel the reference uses), build
    the 9-row im2col matrix per batch (cheap: 29 MB total), shard batch dim
    across 8 cores (8 batches per core).
  - Device (per core): one [128, 3136] moving tile per batch PAIR holds the
    pair's im2col matrix [18, 12544] split into 4 pixel segments placed at
    partition offsets 0/32/64/96 (one contiguous DMA, full port spread).
    Stationary weight [128, 128] is block-diagonal over the pair (cols 0-63
    batch A channels, 64-127 batch B) and replicated at the 4 partition
    offsets. Each segment runs 7 fp32 matmuls (N=448) at tile_position
    (32s, 0) -> PSUM [128, 448]; bias is fused into the PSUM->SBUF
    evacuation (alternating VectorE tensor_scalar_add / ScalarE activation
    Identity), and each segment's [128, 3136] staging tile streams out as a
    1.6 MiB DMA.
"""

import sys

if "/opt/trn_rl_repo" not in sys.path:
    sys.path.insert(0, "/opt/trn_rl_repo")

import numpy as np

B, CIN, COUT, KS = 64, 64, 64, 3
H, W, HP, WP = 112, 112, 114, 114
NPIX = H * W          # 12544
IMG = HP * WP         # 12996
NCORES = 8
BL = B // NCORES      # 8 local batches per core
PAIRS = BL // 2       # 4
KDIM = 2 * KS * KS    # 18
NSEG = 4              # pixel segments per pair (partition offsets 0/32/64/96)
SEGW = NPIX // NSEG   # 3136
NT = 448              # pixels per matmul; 7 * 448 == 3136, fits one PSUM bank
TPS = SEGW // NT      # 7 matmul tiles per segment

_CACHE = {}


def _build_bass():
    import concourse.bass as bass
    import concourse.bacc as bacc
    import concourse.mybir as mybir
    from concourse.tile import TileContext

    f32 = mybir.dt.float32
    f32r = mybir.dt.float32r
    # Bacc (not plain Bass): its compile() runs move_matmul_waits_to_ldweights
    # + generate_event_semaphores, without which walrus rejects any sync wait
    # on a Matmult ("Too many sync wait commands").
    nc = bacc.Bacc("TRN2", target_bir_lowering=False, debug=False)
    mv = nc.declare_dram_parameter("mv", [PAIRS, 128, SEGW], f32r,
                                   isOutput=False)
    w2 = nc.declare_dram_parameter("w2", [128, 128], f32r, isOutput=False)
    b2 = nc.declare_dram_parameter("b2", [128, 1], f32, isOutput=False)
    out = nc.declare_dram_parameter("out", [BL * COUT, NPIX], f32,
                                    isOutput=True)

    with TileContext(nc) as tc:
        with (
            tc.tile_pool(name="consts", bufs=1) as consts,
            tc.tile_pool(name="movp", bufs=2) as movp,
            tc.tile_pool(name="stagep", bufs=10) as stagep,
            tc.tile_pool(name="psump", bufs=8, space="PSUM") as psump,
        ):
            w2_t = consts.tile([128, 128], f32r)
            nc.scalar.dma_start(out=w2_t[:], in_=w2[:])
            b2_t = consts.tile([128, 1], f32)
            nc.sync.dma_start(out=b2_t[:], in_=b2[:])




            tidx = 0
            for pair in range(PAIRS):
                # 32-row groups arrive fully (rows 18-31 zero-filled from
                # host; their weight rows are zero too). Per-seg DMAs let
                # each segment's matmuls start as soon as its rows land.
                mov = movp.tile([128, SEGW + 32], f32r, tag="mov")
                for s4 in range(NSEG):
                    nc.scalar.dma_start(
                        out=mov[32 * s4:32 * (s4 + 1), 0:SEGW],
                        in_=mv[pair, 32 * s4:32 * (s4 + 1), :])

                # t-major emission: consecutive matmuls hit different
                # 32-row groups, so up to 4 run concurrently in the PE array.
                stages = [stagep.tile([128, SEGW], f32, tag="stage",
                                      name=f"stage_{pair}_{s}")
                          for s in range(NSEG)]
                for t in range(TPS):
                    n0 = t * NT
                    for seg in range(NSEG):
                        p0 = 32 * seg
                        ps = psump.tile([128, NT], f32, tag="ps")
                        nc.tensor.matmul(ps[:, :],
                                         w2_t[p0:p0 + KDIM, :],
                                         mov[p0:p0 + KDIM, n0:n0 + NT],
                                         start=True, stop=True,
                                         tile_position=(p0, 0))
                        # PSUM -> SBUF with fused bias add; alternate engines.
                        if tidx % 2 == 0:
                            nc.vector.tensor_scalar_add(
                                stages[seg][:, n0:n0 + NT], ps[:, :],
                                b2_t[:, :])
                        else:
                            nc.scalar.activation(
                                stages[seg][:, n0:n0 + NT], ps[:, :],
                                mybir.ActivationFunctionType.Identity,
                                bias=b2_t[:, :])
                        tidx += 1
                    if t == 3:
                        # first 4 columns-of-448 of every stage are done:
                        # start draining while t=4..6 compute
                        for seg in range(NSEG):
                            nc.sync.dma_start(
                                out=out[pair * 128:(pair + 1) * 128,
                                        seg * SEGW:seg * SEGW + 4 * NT],
                                in_=stages[seg][:, 0:4 * NT])
                for seg in range(NSEG):
                    nc.sync.dma_start(
                        out=out[pair * 128:(pair + 1) * 128,
                                seg * SEGW + 4 * NT:(seg + 1) * SEGW],
                        in_=stages[seg][:, 4 * NT:SEGW])
    nc.compile()
    return nc


def _get_nc():
    if "nc" not in _CACHE:
        _CACHE["nc"] = _build_bass()
    return _CACHE["nc"]


def _prep_inputs(x_padded, weight, bias):
    x = np.asarray(x_padded, dtype=np.float32)
    wt = np.asarray(weight, dtype=np.float32)
    bs = np.asarray(bias, dtype=np.float32)

    xs3 = x[:, -1, :, :]                              # [64, 114, 114]
    win = np.lib.stride_tricks.sliding_window_view(xs3, (KS, KS), axis=(1, 2))
    # [64, 112, 112, 3, 3] -> [64, 9, 12544] with row k = (i, j) shift
    mov_all = win.transpose(0, 3, 4, 1, 2).reshape(B, KS * KS, NPIX)
    # [cores, pairs, 18, NSEG, SEGW] -> [cores, pairs, NSEG, 32, SEGW]
    mov_r = mov_all.reshape(NCORES, PAIRS, KDIM, NSEG, SEGW).transpose(0, 1, 3, 2, 4)
    mov_h = np.zeros((NCORES, PAIRS, NSEG, 32, SEGW), np.float32)
    mov_h[:, :, :, :KDIM, :] = mov_r
    mov_h = mov_h.reshape(NCORES, PAIRS, 128, SEGW)

    wl = np.ascontiguousarray(wt[:, -1, :, :]).reshape(COUT, KS * KS)
    w2 = np.zeros((128, 128), np.float32)
    for s in range(NSEG):
        w2[32 * s: 32 * s + 9, 0:64] = wl.T
        w2[32 * s + 9: 32 * s + 18, 64:128] = wl.T
    b2 = np.tile(bs, 2).reshape(128, 1).astype(np.float32)
    return mov_h, w2, b2


def kernel(x_padded, weight, bias, in_height=112, in_width=112, **_unused):
    from concourse.bass_utils import run_bass_kernel_spmd

    mov_h, w2, b2 = _prep_inputs(x_padded, weight, bias)
    nc = _get_nc()
    in_maps = [
        {"mv": mov_h[c], "w2": w2, "b2": b2}
        for c in range(NCORES)
    ]
    res = run_bass_kernel_spmd(nc, in_maps, core_ids=list(range(NCORES)))
    outs = [
        np.asarray(res.results[c]["out"]).reshape(BL, COUT, H, W)
        for c in range(NCORES)
    ]
    return np.concatenate(outs, axis=0)



# revision 2
# speedup vs baseline: 1.3593x; 1.3593x over previous
"""Trainium2 Bass kernel for nn_CustomConv2D (degenerate conv: only the last
input channel contributes; 3x3 VALID conv -> 64 out channels + bias).

Strategy (v2, fp16):
  - The harness tolerance is rel_err < 2e-2; fp16 end-to-end gives ~3e-4,
    so both the im2col moving data and the OUTPUT are fp16 -> HBM traffic
    halves (in 3.2 MB, out 12.85 MB per core). Host upcasts the result.
  - Host: slice x_padded[:, -1], build the 9-row im2col per batch, shard
    batch dim across 8 cores (8 batches per core).
  - Device (per core): one [128, 3136] fp16 moving tile per batch PAIR:
    the pair's im2col [18, 12544] split into 4 pixel segments at partition
    offsets 0/32/64/96 (one contiguous DMA, full 16-engine port spread).
    Stationary weight [128, 128] fp16 is block-diagonal over the pair and
    replicated at the 4 offsets. Each segment runs 7 matmuls (N=448,
    1 cycle/row in fp16) -> PSUM f32 [128, 448]; bias is fused into the
    PSUM->SBUF evacuation (alternating VectorE / ScalarE, out dtype fp16),
    and stages stream out as fp16 DMAs overlapped with compute.
"""

import sys

if "/opt/trn_rl_repo" not in sys.path:
    sys.path.insert(0, "/opt/trn_rl_repo")

import numpy as np

B, CIN, COUT, KS = 64, 64, 64, 3
H, W, HP, WP = 112, 112, 114, 114
NPIX = H * W          # 12544
IMG = HP * WP         # 12996
NCORES = 8
BL = B // NCORES      # 8 local batches per core
PAIRS = BL // 2       # 4
KDIM = 2 * KS * KS    # 18
NSEG = 4              # pixel segments per pair (partition offsets 0/32/64/96)
SEGW = NPIX // NSEG   # 3136
NT = 448              # pixels per matmul; 7 * 448 == 3136, fits one PSUM bank
TPS = SEGW // NT      # 7 matmul tiles per segment

_CACHE = {}


def _build_bass():
    import concourse.bass as bass
    import concourse.bacc as bacc
    import concourse.mybir as mybir
    from concourse.tile import TileContext

    f32 = mybir.dt.float32
    f16 = mybir.dt.float16
    # Bacc (not plain Bass): its compile() runs move_matmul_waits_to_ldweights
    # + generate_event_semaphores, without which walrus rejects any sync wait
    # on a Matmult ("Too many sync wait commands").
    nc = bacc.Bacc("TRN2", target_bir_lowering=False, debug=False)
    mv = nc.declare_dram_parameter("mv", [PAIRS, 128, SEGW], f16,
                                   isOutput=False)
    w2 = nc.declare_dram_parameter("w2", [128, 128], f16, isOutput=False)
    b2 = nc.declare_dram_parameter("b2", [128, 1], f32, isOutput=False)
    out = nc.declare_dram_parameter("out", [BL * COUT, NPIX], f16,
                                    isOutput=True)

    with TileContext(nc) as tc:
        with (
            tc.tile_pool(name="consts", bufs=1) as consts,
            tc.tile_pool(name="movp", bufs=2) as movp,
            tc.tile_pool(name="stagep", bufs=10) as stagep,
            tc.tile_pool(name="psump", bufs=8, space="PSUM") as psump,
        ):
            w2_t = consts.tile([128, 128], f16)
            nc.scalar.dma_start(out=w2_t[:], in_=w2[:])
            b2_t = consts.tile([128, 1], f32)
            nc.sync.dma_start(out=b2_t[:], in_=b2[:])

            tidx = 0
            for pair in range(PAIRS):
                # 32-row groups arrive fully (rows 18-31 zero-filled from
                # host; their weight rows are zero too). Per-seg DMAs let
                # each segment's matmuls start as soon as its rows land.
                mov = movp.tile([128, SEGW + 32], f16, tag="mov")
                for s4 in range(NSEG):
                    nc.scalar.dma_start(
                        out=mov[32 * s4:32 * (s4 + 1), 0:SEGW],
                        in_=mv[pair, 32 * s4:32 * (s4 + 1), :])

                # t-major emission: consecutive matmuls hit different
                # 32-row groups, so up to 4 run concurrently in the PE array.
                stages = [stagep.tile([128, SEGW], f16, tag="stage",
                                      name=f"stage_{pair}_{s}")
                          for s in range(NSEG)]
                for t in range(TPS):
                    n0 = t * NT
                    for seg in range(NSEG):
                        p0 = 32 * seg
                        ps = psump.tile([128, NT], f32, tag="ps")
                        nc.tensor.matmul(ps[:, :],
                                         w2_t[p0:p0 + KDIM, :],
                                         mov[p0:p0 + KDIM, n0:n0 + NT],
                                         start=True, stop=True,
                                         tile_position=(p0, 0))
                        # PSUM -> SBUF (fp16) with fused bias add; alternate
                        # engines.
                        if tidx % 2 == 0:
                            nc.vector.tensor_scalar_add(
                                stages[seg][:, n0:n0 + NT], ps[:, :],
                                b2_t[:, :])
                        else:
                            nc.scalar.activation(
                                stages[seg][:, n0:n0 + NT], ps[:, :],
                                mybir.ActivationFunctionType.Identity,
                                bias=b2_t[:, :])
                        tidx += 1
                    if t == 3:
                        # first 4 columns-of-448 of every stage are done:
                        # start draining while t=4..6 compute
                        for seg in range(NSEG):
                            nc.sync.dma_start(
                                out=out[pair * 128:(pair + 1) * 128,
                                        seg * SEGW:seg * SEGW + 4 * NT],
                                in_=stages[seg][:, 0:4 * NT])
                for seg in range(NSEG):
                    nc.sync.dma_start(
                        out=out[pair * 128:(pair + 1) * 128,
                                seg * SEGW + 4 * NT:(seg + 1) * SEGW],
                        in_=stages[seg][:, 4 * NT:SEGW])
    nc.compile()
    return nc


def _get_nc():
    if "nc" not in _CACHE:
        _CACHE["nc"] = _build_bass()
    return _CACHE["nc"]


def _prep_inputs(x_padded, weight, bias):
    x = np.asarray(x_padded, dtype=np.float32)
    wt = np.asarray(weight, dtype=np.float32)
    bs = np.asarray(bias, dtype=np.float32)

    xs3 = x[:, -1, :, :]                              # [64, 114, 114]
    win = np.lib.stride_tricks.sliding_window_view(xs3, (KS, KS), axis=(1, 2))
    # [64, 112, 112, 3, 3] -> [64, 9, 12544] with row k = (i, j) shift
    mov_all = win.transpose(0, 3, 4, 1, 2).reshape(B, KS * KS, NPIX)
    # [cores, pairs, 18, NSEG, SEGW] -> [cores, pairs, NSEG, 32, SEGW]
    mov_r = mov_all.reshape(NCORES, PAIRS, KDIM, NSEG, SEGW).transpose(0, 1, 3, 2, 4)
    mov_h = np.zeros((NCORES, PAIRS, NSEG, 32, SEGW), np.float16)
    mov_h[:, :, :, :KDIM, :] = mov_r
    mov_h = mov_h.reshape(NCORES, PAIRS, 128, SEGW)

    wl = np.ascontiguousarray(wt[:, -1, :, :]).reshape(COUT, KS * KS)
    w2 = np.zeros((128, 128), np.float16)
    for s in range(NSEG):
        w2[32 * s: 32 * s + 9, 0:64] = wl.T
        w2[32 * s + 9: 32 * s + 18, 64:128] = wl.T
    b2 = np.tile(bs, 2).reshape(128, 1).astype(np.float32)
    return mov_h, w2, b2


def kernel(x_padded, weight, bias, in_height=112, in_width=112, **_unused):
    from concourse.bass_utils import run_bass_kernel_spmd

    mov_h, w2, b2 = _prep_inputs(x_padded, weight, bias)
    nc = _get_nc()
    in_maps = [
        {"mv": mov_h[c], "w2": w2, "b2": b2}
        for c in range(NCORES)
    ]
    res = run_bass_kernel_spmd(nc, in_maps, core_ids=list(range(NCORES)))
    outs = [
        np.asarray(res.results[c]["out"]).astype(np.float32).reshape(
            BL, COUT, H, W)
        for c in range(NCORES)
    ]
    return np.concatenate(outs, axis=0)


# revision 5
# speedup vs baseline: 1.8935x; 1.3929x over previous
"""Trainium2 Bass kernel for nn_CustomConv2D (degenerate conv: only the last
input channel contributes; 3x3 VALID conv -> 64 out channels + bias).

Strategy (v4):
  - Tolerance is rel_err < 2e-2; fp16 end-to-end gives ~2e-4, so the im2col
    moving data AND the output are fp16 (in 1.81 MB, out 12.85 MB per core;
    host upcasts the result).
  - Host: slice x_padded[:, -1], build the 9-row im2col per batch WITHOUT
    zero padding rows ([pairs, seg, 18, 3136]), shard batch across 8 cores.
  - Input: 16 DMAs (one per pair x segment) on the GpSimd/SWDGE queue so
    the Scalar sequencer stays free for evacuations; rows land directly at
    partition offsets 32s..32s+17.
  - Inner loop: for each t (7 x 448 pixels per segment) the 4 concurrent
    quadrant matmuls (stationary replicated at partition 0/32/64/96,
    block-diagonal over the batch pair) write 4 DISTINCT BANKS of one
    [128, 2048] PSUM tile. A single strided-AP op evacuates all four banks
    with fused bias add, alternating Scalar/Vector engines (7 grouped
    evacs per pair instead of 28 small ones; per-op overhead ~400 ns).
  - Evacs t=0..3 write stageA, t=4..6 write stageB (seg-major layout), so
    each stage drains as ONE large contiguous DMA (1.84 / 1.38 MB) with no
    AP overlap with later evacs -> only 8 output DMAs per core, all big.
"""

import sys

if "/opt/trn_rl_repo" not in sys.path:
    sys.path.insert(0, "/opt/trn_rl_repo")

import numpy as np

B, CIN, COUT, KS = 64, 64, 64, 3
H, W, HP, WP = 112, 112, 114, 114
NPIX = H * W          # 12544
IMG = HP * WP         # 12996
NCORES = 8
BL = B // NCORES      # 8 local batches per core
PAIRS = BL // 2       # 4
KDIM = 2 * KS * KS    # 18
NSEG = 4              # pixel segments per pair (partition offsets 0/32/64/96)
SEGW = NPIX // NSEG   # 3136
NT = 448              # pixels per matmul; 7 * 448 == 3136
TPS = SEGW // NT      # 7 matmul tiles per segment
BANK = 512            # f32 elems per PSUM bank
TA = 4                # t-chunks staged in stageA (drained early)
TB = TPS - TA         # 3 t-chunks in stageB
WA, WB = TA * NT, TB * NT   # 1792, 1344

_CACHE = {}


def _build_bass():
    import bass_rust
    import concourse.bass as bass
    import concourse.bacc as bacc
    import concourse.mybir as mybir
    from concourse.tile import TileContext

    f32 = mybir.dt.float32
    f16 = mybir.dt.float16

    def apx(base_ap, extra_off, dims):
        """Custom access pattern on a tile: dims = [[stride, size], ...]
        (elements), first dim must be the partition dim."""
        return bass_rust.AP(base_ap.tensor, base_ap.offset + extra_off, dims)

    nc = bacc.Bacc("TRN2", target_bir_lowering=False, debug=False)
    mv = nc.declare_dram_parameter("mv", [PAIRS, NSEG, KDIM, SEGW], f16,
                                   isOutput=False)
    w2 = nc.declare_dram_parameter("w2", [128, 128], f16, isOutput=False)
    b2 = nc.declare_dram_parameter("b2", [128, 1], f32, isOutput=False)
    out = nc.declare_dram_parameter("out", [BL * COUT, NPIX], f16,
                                    isOutput=True)

    with TileContext(nc) as tc:
        with (
            tc.tile_pool(name="consts", bufs=1) as consts,
            tc.tile_pool(name="movp", bufs=PAIRS) as movp,
            tc.tile_pool(name="stageap", bufs=2) as stageap,
            tc.tile_pool(name="stagebp", bufs=2) as stagebp,
            tc.tile_pool(name="psump", bufs=2, space="PSUM") as psump,
        ):
            w2_t = consts.tile([128, 128], f16)
            nc.scalar.dma_start(out=w2_t[:], in_=w2[:])
            b2_t = consts.tile([128, 1], f32)
            nc.sync.dma_start(out=b2_t[:], in_=b2[:])

            # Prefetch every pair's moving tile on the SWDGE queue (idle
            # Pool sequencer; per-segment DMAs so the first segment's
            # matmuls start as soon as it lands).
            movs = []
            for pair in range(PAIRS):
                mov = movp.tile([128, SEGW], f16, tag="mov",
                                name=f"mov_{pair}")
                for s4 in range(NSEG):
                    nc.gpsimd.dma_start(
                        out=mov[32 * s4:32 * s4 + KDIM, :],
                        in_=mv[pair, s4])
                movs.append(mov)

            tidx = 0
            for pair in range(PAIRS):
                mov = movs[pair]
                stage_a = stageap.tile([128, NSEG * WA], f16, tag="sa",
                                       name=f"sa_{pair}")
                stage_b = stagebp.tile([128, NSEG * WB], f16, tag="sb",
                                       name=f"sb_{pair}")
                for t in range(TPS):
                    n0 = t * NT
                    pt = psump.tile([128, 4 * BANK], f32, tag="pt")
                    for seg in range(NSEG):
                        p0 = 32 * seg
                        nc.tensor.matmul(pt[:, BANK * seg:BANK * seg + NT],
                                         w2_t[p0:p0 + KDIM, :],
                                         mov[p0:p0 + KDIM, n0:n0 + NT],
                                         start=True, stop=True,
                                         tile_position=(p0, 0))
                    # One grouped PSUM->SBUF evacuation for all 4 banks with
                    # fused bias add; alternate Scalar/Vector (GpSimd cannot
                    # access PSUM per the BIR verifier).
                    in_ap = apx(pt[:], 0, [[4 * BANK, 128], [BANK, NSEG],
                                           [1, NT]])
                    if t < TA:
                        out_ap = apx(stage_a[:], n0,
                                     [[NSEG * WA, 128], [WA, NSEG], [1, NT]])
                    else:
                        out_ap = apx(stage_b[:], n0 - WA,
                                     [[NSEG * WB, 128], [WB, NSEG], [1, NT]])
                    if tidx % 2 == 0:
                        nc.scalar.activation(
                            out_ap, in_ap,
                            mybir.ActivationFunctionType.Identity,
                            bias=b2_t[:, :])
                    else:
                        nc.vector.tensor_scalar_add(out_ap, in_ap,
                                                    b2_t[:, :])
                    tidx += 1
                    if t == TA - 1:
                        # stageA complete: one big early drain (cols
                        # [0, 1792) of every segment).
                        nc.sync.dma_start(
                            out=apx(out[:], pair * 128 * NPIX,
                                    [[NPIX, 128], [SEGW, NSEG], [1, WA]]),
                            in_=stage_a[:])
                nc.sync.dma_start(
                    out=apx(out[:], pair * 128 * NPIX + WA,
                            [[NPIX, 128], [SEGW, NSEG], [1, WB]]),
                    in_=stage_b[:])
    nc.compile()
    return nc


def _get_nc():
    if "nc" not in _CACHE:
        _CACHE["nc"] = _build_bass()
    return _CACHE["nc"]


def _prep_inputs(x_padded, weight, bias):
    x = np.asarray(x_padded, dtype=np.float32)
    wt = np.asarray(weight, dtype=np.float32)
    bs = np.asarray(bias, dtype=np.float32)

    xs3 = x[:, -1, :, :]                              # [64, 114, 114]
    win = np.lib.stride_tricks.sliding_window_view(xs3, (KS, KS), axis=(1, 2))
    # [64, 112, 112, 3, 3] -> [64, 9, 12544] with row k = (i, j) shift
    mov_all = win.transpose(0, 3, 4, 1, 2).reshape(B, KS * KS, NPIX)
    # -> [cores, pairs, 2, 9, NSEG, SEGW] -> [cores, pairs, NSEG, 18, SEGW]
    mov_r = mov_all.reshape(NCORES, PAIRS, 2, KS * KS, NSEG, SEGW)
    mov_h = np.ascontiguousarray(
        mov_r.transpose(0, 1, 4, 2, 3, 5).reshape(
            NCORES, PAIRS, NSEG, KDIM, SEGW)).astype(np.float16)

    wl = np.ascontiguousarray(wt[:, -1, :, :]).reshape(COUT, KS * KS)
    w2 = np.zeros((128, 128), np.float16)
    for s in range(NSEG):
        w2[32 * s: 32 * s + 9, 0:64] = wl.T
        w2[32 * s + 9: 32 * s + 18, 64:128] = wl.T
    b2 = np.tile(bs, 2).reshape(128, 1).astype(np.float32)
    return mov_h, w2, b2


def kernel(x_padded, weight, bias, in_height=112, in_width=112, **_unused):
    from concourse.bass_utils import run_bass_kernel_spmd

    mov_h, w2, b2 = _prep_inputs(x_padded, weight, bias)
    nc = _get_nc()
    in_maps = [
        {"mv": mov_h[c], "w2": w2, "b2": b2}
        for c in range(NCORES)
    ]
    res = run_bass_kernel_spmd(nc, in_maps, core_ids=list(range(NCORES)))
    outs = [
        np.asarray(res.results[c]["out"]).astype(np.float32).reshape(
            BL, COUT, H, W)
        for c in range(NCORES)
    ]
    return np.concatenate(outs, axis=0)
